# revision 53
# baseline (speedup 1.0000x reference)
"""Causal GQA attention (nkv=1) with RoPE + logit softcap, sharded over 8 trn2 cores.

Sharding: core = 2*b + hh  (b = batch 0..3, hh = head-half 0..1).
Each core computes, for its batch b and its 4 query heads:
  q = rope(x @ Wq_h'.T)          (gain/(sqrt(hd)*softcap) folded into Wq on host)
  k = rope(x @ Wk.T), v = x @ Wv.T   (single kv head, shared across its 4 q heads)
  pT[k,q] = exp(softcap*tanh(qT.k) - softcap) * causal_mask   (max-free softmax:
            softcap bounds logits to +-30 so exp never overflows)
  outT_h = (v.T @ pT) / sum_k pT    accumulated in PSUM; denominator via ones-matmul
  partial_out[tok, :] = sum_h outT_h.T @ Wo[:, head cols].T
Host sums the two half-head partials per batch and stacks batches.

v3 (rewrite): ACT(scalar)-engine-minimal schedule. The kernel is jointly
PE/ACT-bound; tanh+exp over every computed score element is the floor. So:
  - diagonal groups use a PACKED score layout: k-block j only computes its
    visible q-columns [128j, 512), packed contiguously -> 1280 instead of
    2048 columns through matmul, tanh and exp.
  - tanh is ONE activation per group reading a 4-bank [128, 2048] fp32 PSUM
    tile (s_pool), exp is one activation -> p4 bf16.
  - softmax denominator: quad-compress p4 on DVE (3 adds) then a single
    ones-matmul per group (PE cost halved vs pair-compress).
  - V projected directly as [tok, hd] (x-tile stationary), no PE transpose.
  - Wo partials DMA'd to DRAM straight from PSUM (no SBUF staging copy).
  - global software pipeline: AV/denominator lag their scores by one task;
    Q/K/V projections and the previous chunk's Wo run as PE filler inside
    each chunk's ACT-bound stretch. DMAs prioritized so the first tanh
    starts ~12us in.
All matmuls bf16 (1 cyc/row); scores accumulate fp32 in PSUM; tanh keeps
fp32 until the bf16 exp output.
"""
import numpy as np
import ml_dtypes

import concourse.bacc as bacc
import concourse.mybir as mybir
import concourse.tile as tile
from concourse.bass_utils import run_bass_kernel_spmd

F32 = mybir.dt.float32
BF16 = mybir.dt.bfloat16
NPBF16 = ml_dtypes.bfloat16

B, T, D = 4, 2048, 1024
NH, NKV, HD = 8, 1, 128
SOFTCAP = 30.0
NHL = 4            # heads per core
CH = 512           # q-chunk size
NCH = T // CH      # 4 chunks
NKT = D // 128     # 8 k-tiles over D
NTT = T // 128     # 16 token tiles

# packed column offsets for diagonal groups: k-block j (visible width
# 512-128j) starts at DOFF[j], arranged so every matmul output stays inside
# one 2KB PSUM bank (512 fp32) with zero padding: bank0 = j0(512),
# bank1 = j1(384) + j3(128), bank2 = j2(256). Total 1280 packed columns.
DOFF = [0, 512, 1024, 896]
NWD = 1280         # diag tanh/exp span
DW = 4 * CH        # score tile width (non-diag)


def _build_nc():
    nc = bacc.Bacc()

    xT = nc.dram_tensor("xT", [D, T], BF16, kind="ExternalInput")
    wqT = nc.dram_tensor("wqT", [D, NHL * HD], BF16, kind="ExternalInput")
    wkT = nc.dram_tensor("wkT", [D, HD], BF16, kind="ExternalInput")
    wvT = nc.dram_tensor("wvT", [D, HD], BF16, kind="ExternalInput")
    woT = nc.dram_tensor("woT", [NHL * HD, D], BF16, kind="ExternalInput")
    cc = nc.dram_tensor("cc", [HD, T], BF16, kind="ExternalInput")
    ssw = nc.dram_tensor("ssw", [HD, T], BF16, kind="ExternalInput")
    tri = nc.dram_tensor("tri", [128, 128], BF16, kind="ExternalInput")
    onesv = nc.dram_tensor("onesv", [128, 128], BF16, kind="ExternalInput")
    out = nc.dram_tensor("out", [T, D], BF16, kind="ExternalOutput")

    xT_t = xT.rearrange("(kt p) t -> p kt t", p=128)      # [128, 8, 2048]
    wqT_t = wqT.rearrange("(kt p) c -> p kt c", p=128)    # [128, 8, 512]
    wkT_t = wkT.rearrange("(kt p) c -> p kt c", p=128)    # [128, 8, 128]
    wvT_t = wvT.rearrange("(kt p) c -> p kt c", p=128)    # [128, 8, 128]
    woT_t = woT.rearrange("(h p) c -> p h c", p=128)      # [128, 4, 1024]

    with tile.TileContext(nc) as tc:
        with (
            tc.tile_pool(name="persist", bufs=1) as persist,
            tc.tile_pool(name="wpool", bufs=1) as wpool,
            tc.tile_pool(name="qt_pool", bufs=2) as qt_pool,
            tc.tile_pool(name="rope_pool", bufs=2) as rope_pool,
            tc.tile_pool(name="p_pool", bufs=5) as p_pool,
            tc.tile_pool(name="pp_pool", bufs=2) as pp_pool,
            tc.tile_pool(name="t4_pool", bufs=1) as t4_pool,
            tc.tile_pool(name="avn_pool", bufs=12) as avn_pool,
            tc.tile_pool(name="osb_pool", bufs=3) as osb_pool,
            tc.tile_pool(name="norm_pool", bufs=2) as norm_pool,
            tc.tile_pool(name="s_pool", bufs=1, space="PSUM") as s_pool,
            tc.tile_pool(name="acc_pool", bufs=1, space="PSUM") as acc_pool,
            tc.tile_pool(name="d_pool", bufs=1, space="PSUM") as d_pool,
            tc.tile_pool(name="pj_pool", bufs=2, space="PSUM") as pj_pool,
        ):
            # --- persistent tiles ---
            wq_sb = wpool.tile([128, NKT, NHL * HD], BF16)
            wk_sb = wpool.tile([128, NKT, HD], BF16)
            wv_sb = wpool.tile([128, NKT, HD], BF16)
            wo_sb = wpool.tile([128, NHL, D], BF16)
            cc_sb = wpool.tile([HD, T], BF16)
            ssw_sb = wpool.tile([HD, T], BF16)
            tri_sb = wpool.tile([128, 128], BF16)
            ones_sb = wpool.tile([128, 128], BF16)
            xT_sb = wpool.tile([128, NKT, T], BF16)
            kT_sb = persist.tile([HD, T], BF16)
            v_sb = persist.tile([128, NTT, HD], BF16)
            # softmax bias is dropped entirely: p' = e^{30 tanh} = e^30 * p
            # scales every row uniformly, which the normalization cancels.
            # e30 feeds the gpsimd pow-based exp offload: pow(e^30, t) = p'.
            e30_sb = persist.tile([128, DW], BF16)
            nc.gpsimd.memset(e30_sb[:], float(np.exp(SOFTCAP)))

            # --- DMA priorities ---
            # the HWDGE queues are issue-rate-bound (~600ns/DMA), so move
            # everything in a few big multi-dim transfers. chunk-0 operands
            # split across both queues; the scalar queue stays SHORT so the
            # scalar engine is free for the first tanh ASAP, and the bulky
            # late-use tensors (cc/ssw rest, wo) ride sync behind x1.
            nc.sync.dma_start(xT_sb[:, 0:4, 0:CH], xT_t[:, 0:4, 0:CH])
            nc.sync.dma_start(wq_sb[:, 0:4, :], wqT_t[:, 0:4, :])
            nc.sync.dma_start(wq_sb[:, 4:8, :], wqT_t[:, 4:8, :])
            nc.scalar.dma_start(xT_sb[:, 4:8, 0:CH], xT_t[:, 4:8, 0:CH])
            nc.scalar.dma_start(wk_sb[:], wkT_t)
            nc.scalar.dma_start(cc_sb[:, 0:CH], cc[:, 0:CH])
            nc.scalar.dma_start(ssw_sb[:, 0:CH], ssw[:, 0:CH])
            nc.scalar.dma_start(wv_sb[:], wvT_t)
            nc.scalar.dma_start(tri_sb[:], tri[:])
            nc.scalar.dma_start(ones_sb[:], onesv[:])
            nc.sync.dma_start(xT_sb[:, :, CH:2 * CH], xT_t[:, :, CH:2 * CH])
            nc.sync.dma_start(cc_sb[:, CH:T], cc[:, CH:T])
            nc.sync.dma_start(ssw_sb[:, CH:T], ssw[:, CH:T])
            nc.sync.dma_start(wo_sb[:], woT_t)
            for c in range(2, NCH):
                nc.sync.dma_start(xT_sb[:, :, c * CH:(c + 1) * CH],
                                  xT_t[:, :, c * CH:(c + 1) * CH])

            def rope_to(dst_ap, src_ps, c, eng=None):
                """dst = rope(src) for a [128, CH] chunk at token offset c*CH.

                All-bf16 after the PSUM read; partition half-swap must go
                through tensor_copy (TT ops need aligned partitions). PSUM
                reads stay on DVE; eng (DVE for latency-critical Q ropes,
                gpsimd for the latency-tolerant K ropes) runs the mul/add."""
                if eng is None:
                    eng = nc.vector
                csl = slice(c * CH, (c + 1) * CH)
                qb = rope_pool.tile([128, CH], BF16, tag="qb", name="qb")
                nc.vector.tensor_copy(qb[:], src_ps[:])
                swp = rope_pool.tile([128, CH], BF16, tag="swp", name="swp")
                nc.vector.tensor_copy(swp[0:64, :], qb[64:128, :])
                nc.vector.tensor_copy(swp[64:128, :], qb[0:64, :])
                m1 = rope_pool.tile([128, CH], BF16, tag="m1", name="m1")
                eng.tensor_mul(m1[:], qb[:], cc_sb[:, csl])
                m2 = rope_pool.tile([128, CH], BF16, tag="m2", name="m2")
                eng.tensor_mul(m2[:], swp[:], ssw_sb[:, csl])
                eng.tensor_add(dst_ap, m1[:], m2[:])

            # ---- filler units (pure-PE work scheduled into ACT-bound gaps) ----
            qt_tiles = {}     # c -> qt tile [HD, NHL, CH]

            def qp_unit(c, h):
                csl = slice(c * CH, (c + 1) * CH)
                if h == 0:
                    qt_tiles[c] = qt_pool.tile([HD, NHL, CH], BF16, tag="qt",
                                               name="qt")
                q_ps = pj_pool.tile([128, CH], F32, tag="pj", name="q_ps")
                for kt in range(NKT):
                    nc.tensor.matmul(
                        q_ps[0:HD, :], wq_sb[:, kt, h * HD:(h + 1) * HD],
                        xT_sb[:, kt, csl], start=(kt == 0), stop=(kt == NKT - 1))
                rope_to(qt_tiles[c][:, h, :], q_ps[0:HD, :], c)

            def kp_unit(c):
                csl = slice(c * CH, (c + 1) * CH)
                k_ps = pj_pool.tile([128, CH], F32, tag="pj", name="k_ps")
                for kt in range(NKT):
                    nc.tensor.matmul(k_ps[0:HD, :], wk_sb[:, kt, :],
                                     xT_sb[:, kt, csl],
                                     start=(kt == 0), stop=(kt == NKT - 1))
                # K rope is emitted a chunk ahead of first use: gpsimd's
                # slowness is hidden, and DVE stays off the critical path
                rope_to(kT_sb[:, csl], k_ps[0:HD, :], c, eng=nc.gpsimd)

            def vp_unit(c, tt):
                # V directly as [tok, hd]: x-tile stationary, wv moving.
                tsl = slice((c * 4 + tt) * 128, (c * 4 + tt + 1) * 128)
                v_ps = pj_pool.tile([128, CH], F32, tag="pj", name="v_ps")
                for kt in range(NKT):
                    nc.tensor.matmul(v_ps[:, 0:HD], xT_sb[:, kt, tsl],
                                     wv_sb[:, kt, :],
                                     start=(kt == 0), stop=(kt == NKT - 1))
                nc.vector.tensor_copy(v_sb[:, c * 4 + tt, :], v_ps[:, 0:HD])

            avn_tiles = {}    # (c, h) -> avn tile

            def wo_unit(c, u):
                tt, dc = u // 2, u % 2
                o_ps = pj_pool.tile([128, CH], F32, tag="pj", name="o_ps")
                for h in range(NHL):
                    nc.tensor.matmul(
                        o_ps[:], avn_tiles[(c, h)][:, tt * 128:(tt + 1) * 128],
                        wo_sb[:, h, dc * CH:(dc + 1) * CH],
                        start=(h == 0), stop=(h == NHL - 1))
                # DMA can't source PSUM (nor can gpsimd): stage on DVE.
                # bf16 halves the out DMA; host sums partials in fp32.
                o_sb = osb_pool.tile([128, CH], BF16, tag="osb", name="o_sb")
                nc.vector.tensor_copy(o_sb[:], o_ps[:])
                nc.sync.dma_start(
                    out[c * CH + tt * 128: c * CH + (tt + 1) * 128,
                        dc * CH:(dc + 1) * CH], o_sb[:])

            # ---- filler scheduling ----
            emitted = set()

            def emit_unit(u):
                if u in emitted:
                    return
                emitted.add(u)
                kind = u[0]
                if kind == "qp":
                    qp_unit(u[1], u[2])
                elif kind == "kp":
                    kp_unit(u[1])
                elif kind == "vp":
                    vp_unit(u[1], u[2])
                elif kind == "wo":
                    wo_unit(u[1], u[2])
                elif kind == "woa":
                    # chunk-3 h0/h1 partial into out2 (host adds it back)
                    tt, dc = u[2] // 2, u[2] % 2
                    wo_unit(u[1], u[2], heads=(0, 1),
                            dst=out2[tt * 128:(tt + 1) * 128,
                                     dc * CH:(dc + 1) * CH])
                elif kind == "wob":
                    wo_unit(u[1], u[2], heads=(2, 3))

            # per-chunk filler lists. Only qp(c+1,0)/kp(c+1) cross chunk
            # boundaries; vp(c) and qp(c,h>=1) stay inside chunk c (forced
            # just-in-time), and Wo shifts late into the ACT-heavy chunks
            # 2 and 3 to match the causal skew of attention work.
            fillers = {}
            fillers[0] = ([("vp", 0, tt) for tt in range(4)]
                          + [("kp", 1), ("qp", 1, 0)])
            fillers[1] = ([("vp", 1, tt) for tt in range(4)]
                          + [("kp", 2), ("qp", 2, 0)])
            fillers[2] = ([("vp", 2, tt) for tt in range(4)]
                          + [("wo", 0, u) for u in range(8)]
                          + [("kp", 3), ("qp", 3, 0)])
            fillers[3] = ([("vp", 3, tt) for tt in range(4)]
                          + [("wo", 1, u) for u in range(8)]
                          + [("wo", 2, u) for u in range(8)])

            # ---- attention task machinery ----
            pend = []       # lagged AV work queue: (c, h, g, p4_tile)
            AV_LAG = 2      # tasks between exp(i) and its AV consumption
            head_acc = {}   # (c, h) -> (av_ps, d_ps), allocated at g == 0

            def emit_av(c, h, g, p4):
                """AV + quad-compress + ones-matmul for task (c,h,g); the
                consuming accumulators live across the head's groups."""
                diag = g == c
                for tt in range(4):
                    emit_unit(("vp", g, tt))
                if g == 0:
                    av_ps = acc_pool.tile([HD, CH], F32, tag="av", name="av_ps")
                    d_ps = d_pool.tile([128, CH], F32, tag="d", name="d_ps")
                    head_acc[(c, h)] = (av_ps, d_ps)
                av_ps, d_ps = head_acc[(c, h)]
                for j in range(4):
                    kb = 4 * g + j
                    if diag:
                        lo, po = 128 * j, DOFF[j]
                        w = CH - lo
                        nc.tensor.matmul(av_ps[:, lo:CH], v_sb[:, kb, :],
                                         p4[:, po:po + w],
                                         start=(kb == 0),
                                         stop=(g == c and j == 3))
                    else:
                        nc.tensor.matmul(av_ps[:], v_sb[:, kb, :],
                                         p4[:, j * CH:(j + 1) * CH],
                                         start=(kb == 0), stop=False)
                # quad-compress for the denominator: 3 adds -> 1 ones-MM
                ppq = pp_pool.tile([128, CH], BF16, tag="ppq", name="ppq")
                if diag:
                    nc.vector.tensor_copy(ppq[:], p4[:, 0:CH])
                    for j in range(1, 4):
                        lo = 128 * j
                        nc.vector.tensor_add(
                            ppq[:, lo:CH], ppq[:, lo:CH],
                            p4[:, DOFF[j]:DOFF[j] + (CH - lo)])
                else:
                    ppa = pp_pool.tile([128, CH], BF16, tag="ppa", name="ppa")
                    nc.vector.tensor_add(ppa[:], p4[:, 0:CH], p4[:, CH:2 * CH])
                    ppb = pp_pool.tile([128, CH], BF16, tag="ppb", name="ppb")
                    nc.vector.tensor_add(ppb[:], p4[:, 2 * CH:3 * CH],
                                         p4[:, 3 * CH:4 * CH])
                    nc.vector.tensor_add(ppq[:], ppa[:], ppb[:])
                nc.tensor.matmul(d_ps[:], ones_sb[:], ppq[:],
                                 start=(g == 0), stop=(g == c))
                if g == c:
                    # head (c,h) complete: normalize
                    dinv = norm_pool.tile([128, CH], F32, tag="dinv",
                                          name="dinv")
                    nc.vector.reciprocal_approx_fast(dinv[:], d_ps[:])
                    avn = avn_pool.tile([HD, CH], BF16, tag="avn", name="avn")
                    nc.vector.tensor_mul(avn[:], av_ps[:], dinv[:])
                    avn_tiles[(c, h)] = avn

            def emit_scores(c, h, g):
                """scores -> tanh -> exp(-> tri mask) for task (c,h,g)."""
                diag = g == c
                emit_unit(("kp", g))
                qt = qt_tiles[c]
                s_t = s_pool.tile([128, DW], F32, tag="s", name="s_t")
                t4 = t4_pool.tile([128, DW], F32, tag="t4", name="t4")
                p4 = p_pool.tile([128, DW], BF16, tag="p4", name="p4")
                if diag:
                    for j in range(4):
                        kb = 4 * g + j
                        lo, po = 128 * j, DOFF[j]
                        w = CH - lo
                        nc.tensor.matmul(
                            s_t[:, po:po + w],
                            kT_sb[:, kb * 128:(kb + 1) * 128],
                            qt[:, h, lo:CH], start=True, stop=True)
                    nw = NWD
                else:
                    for j in range(4):
                        kb = 4 * g + j
                        nc.tensor.matmul(
                            s_t[:, j * CH:(j + 1) * CH],
                            kT_sb[:, kb * 128:(kb + 1) * 128],
                            qt[:, h, :], start=True, stop=True)
                    nw = DW
                nc.scalar.activation(t4[:, 0:nw], s_t[:, 0:nw],
                                     mybir.ActivationFunctionType.Tanh)
                # offload some exps to the otherwise-idle gpsimd in the
                # ACT-heaviest chunks: pow(e^30, t) there == exp(30t) on ACT
                if (not diag) and c >= 2 and g % 2 == 1:
                    nc.gpsimd.tensor_tensor(p4[:, 0:nw], e30_sb[:, 0:nw],
                                            t4[:, 0:nw], mybir.AluOpType.pow)
                else:
                    nc.scalar.activation(p4[:, 0:nw], t4[:, 0:nw],
                                         mybir.ActivationFunctionType.Exp,
                                         scale=SOFTCAP)
                if diag:
                    # mask the four partially-visible 128-col triangles
                    for j in range(4):
                        po = DOFF[j]
                        nc.vector.tensor_mul(p4[:, po:po + 128],
                                             p4[:, po:po + 128], tri_sb[:])
                return p4

            # ---- main schedule ----
            # prologue: just enough for the first task
            emit_unit(("qp", 0, 0))
            emit_unit(("kp", 0))

            for c in range(NCH):
                if c >= 1:
                    # cross-boundary fillers must have landed (kp(c)/qp(c,0))
                    for u in fillers[c - 1]:
                        emit_unit(u)
                flist = fillers[c]
                # drain fillers one task early so chunk boundaries are clean
                ntasks = max(1, NHL * (c + 1) - 1)
                nf = len(flist)
                ti = 0
                for h in range(NHL):
                    emit_unit(("qp", c, h))
                    for g in range(c + 1):
                        p4 = emit_scores(c, h, g)
                        pend.append((c, h, g, p4))
                        if len(pend) > AV_LAG:
                            emit_av(*pend.pop(0))
                        if g == 0 and h + 1 < NHL:
                            # project the next head now: its rope latency
                            # hides under this head's ACT work
                            emit_unit(("qp", c, h + 1))
                        # spread this chunk's fillers evenly across tasks
                        lo = min(nf, (ti * nf) // ntasks)
                        hi = min(nf, ((ti + 1) * nf) // ntasks)
                        for u in flist[lo:hi]:
                            emit_unit(u)
                        ti += 1
            while pend:
                emit_av(*pend.pop(0))
            for u in fillers[NCH - 1]:
                emit_unit(u)
            for u in range(8):
                emit_unit(("wo", NCH - 1, u))

    nc.compile()
    return nc


_CACHED_NC = None


def _get_nc():
    global _CACHED_NC
    if _CACHED_NC is None:
        _CACHED_NC = _build_nc()
    return _CACHED_NC


def _host_inputs(x, Wq, Wk, Wv, Wo, qk_gain, cos, sin):
    """Build the 8 per-core input maps (bf16 matmul operands)."""
    x = np.asarray(x, np.float32)
    Wq = np.asarray(Wq, np.float32)
    Wk = np.asarray(Wk, np.float32)
    Wv = np.asarray(Wv, np.float32)
    Wo = np.asarray(Wo, np.float32)
    qk_gain = np.asarray(qk_gain, np.float32)
    cos = np.asarray(cos, np.float32)
    sin = np.asarray(sin, np.float32)

    scale = 1.0 / (np.sqrt(HD) * SOFTCAP)
    # Fold per-head gain and softcap scale into Wq rows.
    Wq_s = Wq * (qk_gain[:, None].repeat(HD, 1).reshape(NH * HD, 1) * scale)

    wkT = np.ascontiguousarray(Wk.T.astype(NPBF16))
    wvT = np.ascontiguousarray(Wv.T.astype(NPBF16))
    cosT = cos.T  # [64, T]
    sinT = sin.T
    cc = np.ascontiguousarray(np.concatenate([cosT, cosT], 0).astype(NPBF16))
    # m2 = swap(q) * ssw with swap done via copies: ssw = [-sin; sin]
    ssw = np.ascontiguousarray(np.concatenate([-sinT, sinT], 0).astype(NPBF16))

    # triangular mask for the diagonal 128-blocks: tri[kk, qq] = qq >= kk
    kk = np.arange(128)
    tri = (kk[None, :] >= kk[:, None]).astype(NPBF16)
    onesv = np.ones((128, 128), NPBF16)

    xTs = [np.ascontiguousarray(x[b].T.astype(NPBF16)) for b in range(B)]
    in_maps = []
    for core in range(8):
        b, hh = divmod(core, 2)
        h0 = hh * NHL
        wqT = np.ascontiguousarray(
            Wq_s[h0 * HD:(h0 + NHL) * HD, :].T.astype(NPBF16))
        woT = np.ascontiguousarray(
            Wo[:, h0 * HD:(h0 + NHL) * HD].T.astype(NPBF16))
        in_maps.append({
            "xT": xTs[b], "wqT": wqT, "wkT": wkT, "wvT": wvT, "woT": woT,
            "cc": cc, "ssw": ssw, "tri": tri, "onesv": onesv,
        })
    return in_maps


def kernel(x, Wq, Wk, Wv, Wo, qk_gain, cos, sin, _trace=False):
    in_maps = _host_inputs(x, Wq, Wk, Wv, Wo, qk_gain, cos, sin)
    nc = _get_nc()
    res = run_bass_kernel_spmd(nc, in_maps, core_ids=list(range(8)),
                               trace=_trace)
    out = np.empty((B, T, D), np.float32)
    for b in range(B):
        out[b] = (res.results[2 * b]["out"].astype(np.float32)
                  + res.results[2 * b + 1]["out"].astype(np.float32))
    if _trace:
        kernel.last_exec_time_ns = res.exec_time_ns
        kernel.last_results = res
    return out


# revision 55
# speedup vs baseline: 14.3710x; 14.3710x over previous
"""Causal GQA attention (nkv=1) with RoPE + logit softcap, sharded over 8 trn2 cores.

Sharding: core = 2*b + hh  (b = batch 0..3, hh = head-half 0..1).
Each core computes, for its batch b and its 4 query heads:
  q = rope(x @ Wq_h'.T)          (gain/(sqrt(hd)*softcap) folded into Wq on host)
  k = rope(x @ Wk.T), v = x @ Wv.T   (single kv head, shared across its 4 q heads)
  pT[k,q] = exp(softcap*tanh(qT.k) - softcap) * causal_mask   (max-free softmax:
            softcap bounds logits to +-30 so exp never overflows)
  outT_h = (v.T @ pT) / sum_k pT    accumulated in PSUM; denominator via ones-matmul
  partial_out[tok, :] = sum_h outT_h.T @ Wo[:, head cols].T
Host sums the two half-head partials per batch and stacks batches.

v3 (rewrite): ACT(scalar)-engine-minimal schedule. The kernel is jointly
PE/ACT-bound; tanh+exp over every computed score element is the floor. So:
  - diagonal groups use a PACKED score layout: k-block j only computes its
    visible q-columns [128j, 512), packed contiguously -> 1280 instead of
    2048 columns through matmul, tanh and exp.
  - tanh is ONE activation per group reading a 4-bank [128, 2048] fp32 PSUM
    tile (s_pool), exp is one activation -> p4 bf16.
  - softmax denominator: quad-compress p4 on DVE (3 adds) then a single
    ones-matmul per group (PE cost halved vs pair-compress).
  - V projected directly as [tok, hd] (x-tile stationary), no PE transpose.
  - Wo partials DMA'd to DRAM straight from PSUM (no SBUF staging copy).
  - global software pipeline: AV/denominator lag their scores by one task;
    Q/K/V projections and the previous chunk's Wo run as PE filler inside
    each chunk's ACT-bound stretch. DMAs prioritized so the first tanh
    starts ~12us in.
All matmuls bf16 (1 cyc/row); scores accumulate fp32 in PSUM; tanh keeps
fp32 until the bf16 exp output.
"""
import numpy as np
import ml_dtypes

import concourse.bacc as bacc
import concourse.mybir as mybir
import concourse.tile as tile
from concourse.bass_utils import run_bass_kernel_spmd

F32 = mybir.dt.float32
BF16 = mybir.dt.bfloat16
NPBF16 = ml_dtypes.bfloat16

B, T, D = 4, 2048, 1024
NH, NKV, HD = 8, 1, 128
SOFTCAP = 30.0
NHL = 4            # heads per core
CH = 512           # q-chunk size
NCH = T // CH      # 4 chunks
NKT = D // 128     # 8 k-tiles over D
NTT = T // 128     # 16 token tiles

# packed column offsets for diagonal groups: k-block j (visible width
# 512-128j) starts at DOFF[j], arranged so every matmul output stays inside
# one 2KB PSUM bank (512 fp32) with zero padding: bank0 = j0(512),
# bank1 = j1(384) + j3(128), bank2 = j2(256). Total 1280 packed columns.
DOFF = [0, 512, 1024, 896]
NWD = 1280         # diag tanh/exp span
DW = 4 * CH        # score tile width (non-diag)


def _build_nc():
    nc = bacc.Bacc()

    xT = nc.dram_tensor("xT", [D, T], BF16, kind="ExternalInput")
    wqT = nc.dram_tensor("wqT", [D, NHL * HD], BF16, kind="ExternalInput")
    wkT = nc.dram_tensor("wkT", [D, HD], BF16, kind="ExternalInput")
    wvT = nc.dram_tensor("wvT", [D, HD], BF16, kind="ExternalInput")
    woT = nc.dram_tensor("woT", [NHL * HD, D], BF16, kind="ExternalInput")
    cc = nc.dram_tensor("cc", [HD, T], BF16, kind="ExternalInput")
    ssw = nc.dram_tensor("ssw", [HD, T], BF16, kind="ExternalInput")
    tri = nc.dram_tensor("tri", [128, 128], BF16, kind="ExternalInput")
    onesv = nc.dram_tensor("onesv", [128, 128], BF16, kind="ExternalInput")
    out = nc.dram_tensor("out", [T, D], BF16, kind="ExternalOutput")

    xT_t = xT.rearrange("(kt p) t -> p kt t", p=128)      # [128, 8, 2048]
    wqT_t = wqT.rearrange("(kt p) c -> p kt c", p=128)    # [128, 8, 512]
    wkT_t = wkT.rearrange("(kt p) c -> p kt c", p=128)    # [128, 8, 128]
    wvT_t = wvT.rearrange("(kt p) c -> p kt c", p=128)    # [128, 8, 128]
    woT_t = woT.rearrange("(h p) c -> p h c", p=128)      # [128, 4, 1024]

    with tile.TileContext(nc) as tc:
        with (
            tc.tile_pool(name="persist", bufs=1) as persist,
            tc.tile_pool(name="wpool", bufs=1) as wpool,
            tc.tile_pool(name="qt_pool", bufs=2) as qt_pool,
            tc.tile_pool(name="rope_pool", bufs=2) as rope_pool,
            tc.tile_pool(name="p_pool", bufs=5) as p_pool,
            tc.tile_pool(name="pp_pool", bufs=2) as pp_pool,
            tc.tile_pool(name="t4_pool", bufs=1) as t4_pool,
            tc.tile_pool(name="avn_pool", bufs=12) as avn_pool,
            tc.tile_pool(name="osb_pool", bufs=3) as osb_pool,
            tc.tile_pool(name="norm_pool", bufs=2) as norm_pool,
            tc.tile_pool(name="s_pool", bufs=1, space="PSUM") as s_pool,
            tc.tile_pool(name="acc_pool", bufs=1, space="PSUM") as acc_pool,
            tc.tile_pool(name="d_pool", bufs=1, space="PSUM") as d_pool,
            tc.tile_pool(name="pj_pool", bufs=2, space="PSUM") as pj_pool,
        ):
            # --- persistent tiles ---
            wq_sb = wpool.tile([128, NKT, NHL * HD], BF16)
            wk_sb = wpool.tile([128, NKT, HD], BF16)
            wv_sb = wpool.tile([128, NKT, HD], BF16)
            wo_sb = wpool.tile([128, NHL, D], BF16)
            cc_sb = wpool.tile([HD, T], BF16)
            ssw_sb = wpool.tile([HD, T], BF16)
            tri_sb = wpool.tile([128, 128], BF16)
            ones_sb = wpool.tile([128, 128], BF16)
            xT_sb = wpool.tile([128, NKT, T], BF16)
            kT_sb = persist.tile([HD, T], BF16)
            v_sb = persist.tile([128, NTT, HD], BF16)
            # softmax bias is dropped entirely: p' = e^{30 tanh} = e^30 * p
            # scales every row uniformly, which the normalization cancels.

            # --- DMA priorities ---
            # the HWDGE queues are issue-rate-bound (~600ns/DMA), so move
            # everything in a few big multi-dim transfers. chunk-0 operands
            # split across both queues; the scalar queue stays SHORT so the
            # scalar engine is free for the first tanh ASAP, and the bulky
            # late-use tensors (cc/ssw rest, wo) ride sync behind x1.
            nc.sync.dma_start(xT_sb[:, 0:4, 0:CH], xT_t[:, 0:4, 0:CH])
            nc.sync.dma_start(wq_sb[:, 0:4, :], wqT_t[:, 0:4, :])
            nc.sync.dma_start(wq_sb[:, 4:8, :], wqT_t[:, 4:8, :])
            nc.scalar.dma_start(xT_sb[:, 4:8, 0:CH], xT_t[:, 4:8, 0:CH])
            nc.scalar.dma_start(wk_sb[:], wkT_t)
            nc.scalar.dma_start(cc_sb[:, 0:CH], cc[:, 0:CH])
            nc.scalar.dma_start(ssw_sb[:, 0:CH], ssw[:, 0:CH])
            nc.scalar.dma_start(wv_sb[:], wvT_t)
            nc.scalar.dma_start(tri_sb[:], tri[:])
            nc.scalar.dma_start(ones_sb[:], onesv[:])
            nc.sync.dma_start(xT_sb[:, :, CH:2 * CH], xT_t[:, :, CH:2 * CH])
            nc.sync.dma_start(cc_sb[:, CH:T], cc[:, CH:T])
            nc.sync.dma_start(ssw_sb[:, CH:T], ssw[:, CH:T])
            nc.sync.dma_start(wo_sb[:], woT_t)
            for c in range(2, NCH):
                nc.sync.dma_start(xT_sb[:, :, c * CH:(c + 1) * CH],
                                  xT_t[:, :, c * CH:(c + 1) * CH])

            def rope_to(dst_ap, src_ps, c, eng=None):
                """dst = rope(src) for a [128, CH] chunk at token offset c*CH.

                All-bf16 after the PSUM read; partition half-swap must go
                through tensor_copy (TT ops need aligned partitions). PSUM
                reads stay on DVE; eng (DVE for latency-critical Q ropes,
                gpsimd for the latency-tolerant K ropes) runs the mul/add."""
                if eng is None:
                    eng = nc.vector
                csl = slice(c * CH, (c + 1) * CH)
                qb = rope_pool.tile([128, CH], BF16, tag="qb", name="qb")
                nc.vector.tensor_copy(qb[:], src_ps[:])
                swp = rope_pool.tile([128, CH], BF16, tag="swp", name="swp")
                nc.vector.tensor_copy(swp[0:64, :], qb[64:128, :])
                nc.vector.tensor_copy(swp[64:128, :], qb[0:64, :])
                m1 = rope_pool.tile([128, CH], BF16, tag="m1", name="m1")
                eng.tensor_mul(m1[:], qb[:], cc_sb[:, csl])
                m2 = rope_pool.tile([128, CH], BF16, tag="m2", name="m2")
                eng.tensor_mul(m2[:], swp[:], ssw_sb[:, csl])
                eng.tensor_add(dst_ap, m1[:], m2[:])

            # ---- filler units (pure-PE work scheduled into ACT-bound gaps) ----
            qt_tiles = {}     # c -> qt tile [HD, NHL, CH]

            def qp_unit(c, h):
                csl = slice(c * CH, (c + 1) * CH)
                if h == 0:
                    qt_tiles[c] = qt_pool.tile([HD, NHL, CH], BF16, tag="qt",
                                               name="qt")
                q_ps = pj_pool.tile([128, CH], F32, tag="pj", name="q_ps")
                for kt in range(NKT):
                    nc.tensor.matmul(
                        q_ps[0:HD, :], wq_sb[:, kt, h * HD:(h + 1) * HD],
                        xT_sb[:, kt, csl], start=(kt == 0), stop=(kt == NKT - 1))
                rope_to(qt_tiles[c][:, h, :], q_ps[0:HD, :], c)

            def kp_unit(c):
                csl = slice(c * CH, (c + 1) * CH)
                k_ps = pj_pool.tile([128, CH], F32, tag="pj", name="k_ps")
                for kt in range(NKT):
                    nc.tensor.matmul(k_ps[0:HD, :], wk_sb[:, kt, :],
                                     xT_sb[:, kt, csl],
                                     start=(kt == 0), stop=(kt == NKT - 1))
                # K rope is emitted a chunk ahead of first use: gpsimd's
                # slowness is hidden, and DVE stays off the critical path
                rope_to(kT_sb[:, csl], k_ps[0:HD, :], c, eng=nc.gpsimd)

            def vp_unit(c, tt):
                # V directly as [tok, hd]: x-tile stationary, wv moving.
                tsl = slice((c * 4 + tt) * 128, (c * 4 + tt + 1) * 128)
                v_ps = pj_pool.tile([128, CH], F32, tag="pj", name="v_ps")
                for kt in range(NKT):
                    nc.tensor.matmul(v_ps[:, 0:HD], xT_sb[:, kt, tsl],
                                     wv_sb[:, kt, :],
                                     start=(kt == 0), stop=(kt == NKT - 1))
                nc.vector.tensor_copy(v_sb[:, c * 4 + tt, :], v_ps[:, 0:HD])

            avn_tiles = {}    # (c, h) -> avn tile

            def wo_unit(c, u):
                tt, dc = u // 2, u % 2
                o_ps = pj_pool.tile([128, CH], F32, tag="pj", name="o_ps")
                for h in range(NHL):
                    nc.tensor.matmul(
                        o_ps[:], avn_tiles[(c, h)][:, tt * 128:(tt + 1) * 128],
                        wo_sb[:, h, dc * CH:(dc + 1) * CH],
                        start=(h == 0), stop=(h == NHL - 1))
                # DMA can't source PSUM (nor can gpsimd): stage on DVE.
                # bf16 halves the out DMA; host sums partials in fp32.
                o_sb = osb_pool.tile([128, CH], BF16, tag="osb", name="o_sb")
                nc.vector.tensor_copy(o_sb[:], o_ps[:])
                nc.sync.dma_start(
                    out[c * CH + tt * 128: c * CH + (tt + 1) * 128,
                        dc * CH:(dc + 1) * CH], o_sb[:])

            # ---- filler scheduling ----
            emitted = set()

            def emit_unit(u):
                if u in emitted:
                    return
                emitted.add(u)
                kind = u[0]
                if kind == "qp":
                    qp_unit(u[1], u[2])
                elif kind == "kp":
                    kp_unit(u[1])
                elif kind == "vp":
                    vp_unit(u[1], u[2])
                elif kind == "wo":
                    wo_unit(u[1], u[2])
                elif kind == "woa":
                    # chunk-3 h0/h1 partial into out2 (host adds it back)
                    tt, dc = u[2] // 2, u[2] % 2
                    wo_unit(u[1], u[2], heads=(0, 1),
                            dst=out2[tt * 128:(tt + 1) * 128,
                                     dc * CH:(dc + 1) * CH])
                elif kind == "wob":
                    wo_unit(u[1], u[2], heads=(2, 3))

            # per-chunk filler lists. Only qp(c+1,0)/kp(c+1) cross chunk
            # boundaries; vp(c) and qp(c,h>=1) stay inside chunk c (forced
            # just-in-time), and Wo shifts late into the ACT-heavy chunks
            # 2 and 3 to match the causal skew of attention work.
            fillers = {}
            fillers[0] = ([("vp", 0, tt) for tt in range(4)]
                          + [("kp", 1), ("qp", 1, 0)])
            fillers[1] = ([("vp", 1, tt) for tt in range(4)]
                          + [("kp", 2), ("qp", 2, 0)])
            fillers[2] = ([("vp", 2, tt) for tt in range(4)]
                          + [("wo", 0, u) for u in range(8)]
                          + [("kp", 3), ("qp", 3, 0)])
            fillers[3] = ([("vp", 3, tt) for tt in range(4)]
                          + [("wo", 1, u) for u in range(8)]
                          + [("wo", 2, u) for u in range(8)])

            # ---- attention task machinery ----
            pend = []       # lagged AV work queue: (c, h, g, p4_tile)
            AV_LAG = 2      # tasks between exp(i) and its AV consumption
            head_acc = {}   # (c, h) -> (av_ps, d_ps), allocated at g == 0

            def emit_av(c, h, g, p4):
                """AV + quad-compress + ones-matmul for task (c,h,g); the
                consuming accumulators live across the head's groups."""
                diag = g == c
                for tt in range(4):
                    emit_unit(("vp", g, tt))
                if g == 0:
                    av_ps = acc_pool.tile([HD, CH], F32, tag="av", name="av_ps")
                    d_ps = d_pool.tile([128, CH], F32, tag="d", name="d_ps")
                    head_acc[(c, h)] = (av_ps, d_ps)
                av_ps, d_ps = head_acc[(c, h)]
                for j in range(4):
                    kb = 4 * g + j
                    if diag:
                        lo, po = 128 * j, DOFF[j]
                        w = CH - lo
                        nc.tensor.matmul(av_ps[:, lo:CH], v_sb[:, kb, :],
                                         p4[:, po:po + w],
                                         start=(kb == 0),
                                         stop=(g == c and j == 3))
                    else:
                        nc.tensor.matmul(av_ps[:], v_sb[:, kb, :],
                                         p4[:, j * CH:(j + 1) * CH],
                                         start=(kb == 0), stop=False)
                # quad-compress for the denominator: 3 adds -> 1 ones-MM
                ppq = pp_pool.tile([128, CH], BF16, tag="ppq", name="ppq")
                if diag:
                    nc.vector.tensor_copy(ppq[:], p4[:, 0:CH])
                    for j in range(1, 4):
                        lo = 128 * j
                        nc.vector.tensor_add(
                            ppq[:, lo:CH], ppq[:, lo:CH],
                            p4[:, DOFF[j]:DOFF[j] + (CH - lo)])
                else:
                    ppa = pp_pool.tile([128, CH], BF16, tag="ppa", name="ppa")
                    nc.vector.tensor_add(ppa[:], p4[:, 0:CH], p4[:, CH:2 * CH])
                    ppb = pp_pool.tile([128, CH], BF16, tag="ppb", name="ppb")
                    nc.vector.tensor_add(ppb[:], p4[:, 2 * CH:3 * CH],
                                         p4[:, 3 * CH:4 * CH])
                    nc.vector.tensor_add(ppq[:], ppa[:], ppb[:])
                nc.tensor.matmul(d_ps[:], ones_sb[:], ppq[:],
                                 start=(g == 0), stop=(g == c))
                if g == c:
                    # head (c,h) complete: normalize
                    dinv = norm_pool.tile([128, CH], F32, tag="dinv",
                                          name="dinv")
                    nc.vector.reciprocal_approx_fast(dinv[:], d_ps[:])
                    avn = avn_pool.tile([HD, CH], BF16, tag="avn", name="avn")
                    nc.vector.tensor_mul(avn[:], av_ps[:], dinv[:])
                    avn_tiles[(c, h)] = avn

            def emit_scores(c, h, g):
                """scores -> tanh -> exp(-> tri mask) for task (c,h,g)."""
                diag = g == c
                emit_unit(("kp", g))
                qt = qt_tiles[c]
                s_t = s_pool.tile([128, DW], F32, tag="s", name="s_t")
                t4 = t4_pool.tile([128, DW], F32, tag="t4", name="t4")
                p4 = p_pool.tile([128, DW], BF16, tag="p4", name="p4")
                if diag:
                    for j in range(4):
                        kb = 4 * g + j
                        lo, po = 128 * j, DOFF[j]
                        w = CH - lo
                        nc.tensor.matmul(
                            s_t[:, po:po + w],
                            kT_sb[:, kb * 128:(kb + 1) * 128],
                            qt[:, h, lo:CH], start=True, stop=True)
                    nw = NWD
                else:
                    for j in range(4):
                        kb = 4 * g + j
                        nc.tensor.matmul(
                            s_t[:, j * CH:(j + 1) * CH],
                            kT_sb[:, kb * 128:(kb + 1) * 128],
                            qt[:, h, :], start=True, stop=True)
                    nw = DW
                nc.scalar.activation(t4[:, 0:nw], s_t[:, 0:nw],
                                     mybir.ActivationFunctionType.Tanh)
                nc.scalar.activation(p4[:, 0:nw], t4[:, 0:nw],
                                     mybir.ActivationFunctionType.Exp,
                                     scale=SOFTCAP)
                if diag:
                    # mask the four partially-visible 128-col triangles
                    for j in range(4):
                        po = DOFF[j]
                        nc.vector.tensor_mul(p4[:, po:po + 128],
                                             p4[:, po:po + 128], tri_sb[:])
                return p4

            # ---- main schedule ----
            # prologue: just enough for the first task
            emit_unit(("qp", 0, 0))
            emit_unit(("kp", 0))

            for c in range(NCH):
                if c >= 1:
                    # cross-boundary fillers must have landed (kp(c)/qp(c,0))
                    for u in fillers[c - 1]:
                        emit_unit(u)
                flist = fillers[c]
                # drain fillers one task early so chunk boundaries are clean
                ntasks = max(1, NHL * (c + 1) - 1)
                nf = len(flist)
                ti = 0
                for h in range(NHL):
                    emit_unit(("qp", c, h))
                    for g in range(c + 1):
                        p4 = emit_scores(c, h, g)
                        pend.append((c, h, g, p4))
                        if len(pend) > AV_LAG:
                            emit_av(*pend.pop(0))
                        if g == 0 and h + 1 < NHL:
                            # project the next head now: its rope latency
                            # hides under this head's ACT work
                            emit_unit(("qp", c, h + 1))
                        # spread this chunk's fillers evenly across tasks
                        lo = min(nf, (ti * nf) // ntasks)
                        hi = min(nf, ((ti + 1) * nf) // ntasks)
                        for u in flist[lo:hi]:
                            emit_unit(u)
                        ti += 1
            while pend:
                emit_av(*pend.pop(0))
            for u in fillers[NCH - 1]:
                emit_unit(u)
            for u in range(8):
                emit_unit(("wo", NCH - 1, u))

    nc.compile()
    return nc


_CACHED_NC = None


def _get_nc():
    global _CACHED_NC
    if _CACHED_NC is None:
        _CACHED_NC = _build_nc()
    return _CACHED_NC


def _host_inputs(x, Wq, Wk, Wv, Wo, qk_gain, cos, sin):
    """Build the 8 per-core input maps (bf16 matmul operands)."""
    x = np.asarray(x, np.float32)
    Wq = np.asarray(Wq, np.float32)
    Wk = np.asarray(Wk, np.float32)
    Wv = np.asarray(Wv, np.float32)
    Wo = np.asarray(Wo, np.float32)
    qk_gain = np.asarray(qk_gain, np.float32)
    cos = np.asarray(cos, np.float32)
    sin = np.asarray(sin, np.float32)

    scale = 1.0 / (np.sqrt(HD) * SOFTCAP)
    # Fold per-head gain and softcap scale into Wq rows.
    Wq_s = Wq * (qk_gain[:, None].repeat(HD, 1).reshape(NH * HD, 1) * scale)

    wkT = np.ascontiguousarray(Wk.T.astype(NPBF16))
    wvT = np.ascontiguousarray(Wv.T.astype(NPBF16))
    cosT = cos.T  # [64, T]
    sinT = sin.T
    cc = np.ascontiguousarray(np.concatenate([cosT, cosT], 0).astype(NPBF16))
    # m2 = swap(q) * ssw with swap done via copies: ssw = [-sin; sin]
    ssw = np.ascontiguousarray(np.concatenate([-sinT, sinT], 0).astype(NPBF16))

    # triangular mask for the diagonal 128-blocks: tri[kk, qq] = qq >= kk
    kk = np.arange(128)
    tri = (kk[None, :] >= kk[:, None]).astype(NPBF16)
    onesv = np.ones((128, 128), NPBF16)

    xTs = [np.ascontiguousarray(x[b].T.astype(NPBF16)) for b in range(B)]
    in_maps = []
    for core in range(8):
        b, hh = divmod(core, 2)
        h0 = hh * NHL
        wqT = np.ascontiguousarray(
            Wq_s[h0 * HD:(h0 + NHL) * HD, :].T.astype(NPBF16))
        woT = np.ascontiguousarray(
            Wo[:, h0 * HD:(h0 + NHL) * HD].T.astype(NPBF16))
        in_maps.append({
            "xT": xTs[b], "wqT": wqT, "wkT": wkT, "wvT": wvT, "woT": woT,
            "cc": cc, "ssw": ssw, "tri": tri, "onesv": onesv,
        })
    return in_maps


def kernel(x, Wq, Wk, Wv, Wo, qk_gain, cos, sin, _trace=False):
    in_maps = _host_inputs(x, Wq, Wk, Wv, Wo, qk_gain, cos, sin)
    nc = _get_nc()
    res = run_bass_kernel_spmd(nc, in_maps, core_ids=list(range(8)),
                               trace=_trace)
    out = np.empty((B, T, D), np.float32)
    for b in range(B):
        out[b] = (res.results[2 * b]["out"].astype(np.float32)
                  + res.results[2 * b + 1]["out"].astype(np.float32))
    if _trace:
        kernel.last_exec_time_ns = res.exec_time_ns
        kernel.last_results = res
    return out
